# revision 22
# baseline (speedup 1.0000x reference)
"""Multi-head attention (B=2, S=2048, D=1024, H=16, d_head=64) on 8 TRN2 cores.

Sharding: 2-way data parallel over batch x 4-way tensor parallel over heads.
Core c: batch g = c//4, heads [4r, 4r+4) with r = c%4. Each core projects
Q/K/V for its 4 heads from its batch's (pre-transposed) activations, runs
attention for its heads, then computes its row-parallel partial of the Wout
projection (contraction over its own 256 head dims, full 1024 output dims).
The host unshards by summing the 4 partials of each batch group (the
all-reduce of the row-parallel sharding) -- no device collectives at all.

Schedule (v4): the kernel is paced by the ScalarE exp stream (16.8M exps per
core ~ 143us). Everything else is arranged to hide under it:
  - Q proj (chunk 0) and K proj (chunked (et,qc), dt-inner) complete
    incrementally so the first scores/exp fire ~25us in.
  - V proj, Q proj chunks 1-3, and the Wout partials are emitted at demoted
    scheduler priority: the Tile list scheduler slots them into PE gaps.
    PV lags behind exp via a deep exp ring until V tiles land.
  - PV stationary is [V_h (64 cols) | ones (64 cols)]: the softmax
    denominator lands pre-broadcast on PSUM partitions 64-127 in f32;
    normalization = two aligned copies + reciprocal_approx_fast + multiply.
"""

import os
import sys

import numpy as np

for _p in ("/opt/trn_rl_repo",):
    if _p not in sys.path and os.path.isdir(_p):
        sys.path.append(_p)

import ml_dtypes

import concourse.bacc as bacc
import concourse.mybir as mybir
from concourse.bass_utils import run_bass_kernel_spmd
from concourse.tile import TileContext

P = 128
B, S, DM = 2, 2048, 1024
NH_TOT, EH = 16, 64  # total heads, head dim
NCORES = 8
GROUPS = 2  # batch groups of 4 cores
NH = 4  # heads per core
EHC = NH * EH  # 256: head-concat width per core
NDT = DM // P  # 8 d-tiles
NKT = S // P  # 16 key tiles
QC = 512  # q chunk
NQC = S // QC  # 4
VW = P  # V block width: 64 V cols + 64 ones cols (den broadcast)
DEMOTE = 10_000_000  # scheduler priority offset for gap-filler work

BF = mybir.dt.bfloat16
F32 = mybir.dt.float32

_cached_nc = None


def build_nc():
    nc = bacc.Bacc("TRN2", target_bir_lowering=False, debug=False, num_devices=NCORES)

    xqt = nc.declare_dram_parameter("xqt", [DM, S], BF, isOutput=False)
    xkt = nc.declare_dram_parameter("xkt", [DM, S], BF, isOutput=False)
    xvt = nc.declare_dram_parameter("xvt", [DM, S], BF, isOutput=False)
    # weights arrive host-rearranged to partition-major so loads are contiguous
    wqt = nc.declare_dram_parameter("wqt", [P, NDT * EHC], BF, isOutput=False)
    wkt = nc.declare_dram_parameter("wkt", [P, NDT * EHC], BF, isOutput=False)
    wvt = nc.declare_dram_parameter("wvt", [P, NDT * EHC], BF, isOutput=False)
    # Wout rows for this core's 256 head dims, all 1024 output dims
    wo3t = nc.declare_dram_parameter("wo3t", [P, 2 * DM], BF, isOutput=False)
    # row-parallel partial of out^T; host sums the 4 partials per group
    outt = nc.declare_dram_parameter("outt", [DM, S], F32, isOutput=True)

    with TileContext(nc) as tc:
        with (
            tc.tile_pool(name="persist", bufs=1) as persist,
            tc.tile_pool(name="xkp", bufs=8) as xkp,
            tc.tile_pool(name="xqp", bufs=8) as xqp,
            tc.tile_pool(name="xvp", bufs=8) as xvp,
        ):
            # --- persistent SBUF ---
            wq_sb = persist.tile([P, NDT, EHC], BF)
            wk_sb = persist.tile([P, NDT, EHC], BF)
            wv_sb = persist.tile([P, NDT, EHC], BF)
            wo3_sb = persist.tile([P, 2, DM], BF)
            qt_sb = [persist.tile([P, S], BF, name=f"qt{et}") for et in range(2)]
            kt_sb = [persist.tile([P, S], BF, name=f"kt{et}") for et in range(2)]
            v_sb = persist.tile([P, NKT * NH * VW], BF)
            nc.vector.memset(v_sb[:], 1.0)  # ones cols; V data overwrites 0:64

            # Input DMA runs at the ~358 GB/s HBM cap, so arrival ORDER is what
            # matters: only wk/wq + the qc0 slices of xk/xq (~3.5 MB) gate the
            # first exp. xk/xq load in 512-col slices, prioritized qc0 first;
            # later K/Q chunks and xv stream in behind while attention runs.
            # The scalar (ACT) queue only carries early loads -- a DMA occupies
            # its issuing engine until the transfer completes, and scalar must
            # be free once the first exp issues (~14us).
            xk = [xkp.tile([P, S], BF, name=f"xk{dt}", tag="xk") for dt in range(NDT)]
            xv = [xvp.tile([P, S], BF, name=f"xv{dt}", tag="xv") for dt in range(NDT)]
            xq = [xqp.tile([P, S], BF, name=f"xq{dt}", tag="xq") for dt in range(NDT)]

            def _slice_loads(x_tiles, x_par, qc, engs):
                for dt in range(NDT):
                    engs[dt % len(engs)].dma_start(
                        x_tiles[dt][:, qc * QC : (qc + 1) * QC],
                        x_par[dt * P : (dt + 1) * P, qc * QC : (qc + 1) * QC],
                    )

            three = [nc.sync, nc.scalar, nc.gpsimd]
            two = [nc.sync, nc.gpsimd]
            nc.scalar.dma_start(wk_sb[:].rearrange("p d e -> p (d e)"), wkt[:])
            nc.gpsimd.dma_start(wq_sb[:].rearrange("p d e -> p (d e)"), wqt[:])
            _slice_loads(xk, xkt, 0, three)
            _slice_loads(xq, xqt, 0, three)
            nc.sync.dma_start(wv_sb[:].rearrange("p d e -> p (d e)"), wvt[:])
            _slice_loads(xk, xkt, 1, two)
            for dt in range(2):
                two[dt % 2].dma_start(xv[dt][:], xvt[dt * P : (dt + 1) * P, :])
            _slice_loads(xk, xkt, 2, two)
            _slice_loads(xk, xkt, 3, two)
            for dt in range(2, NDT):
                two[dt % 2].dma_start(xv[dt][:], xvt[dt * P : (dt + 1) * P, :])
            for qc in range(1, NQC):
                _slice_loads(xq, xqt, qc, two)
            nc.gpsimd.dma_start(wo3_sb[:].rearrange("p d e -> p (d e)"), wo3t[:])

            with (
                tc.tile_pool(name="pvp", bufs=1, space="PSUM") as pvp,
                tc.tile_pool(name="vp", bufs=1, space="PSUM") as vp,
                tc.tile_pool(name="auxp", bufs=1, space="PSUM") as auxp,
                tc.tile_pool(name="scorep", bufs=2, space="PSUM") as scorep,
                tc.tile_pool(name="exps", bufs=18) as expp,
                tc.tile_pool(name="normp", bufs=2) as normp,
                tc.tile_pool(name="hcp", bufs=4) as hcp,
                tc.tile_pool(name="outstp", bufs=4) as outstp,
            ):

                def emit_qproj(qc, et):
                    # one e-tile chunk of Q proj: 8 matmuls into 1 PSUM bank
                    qp = auxp.tile([P, QC], F32, name="qp", tag="aux")
                    for dt in range(NDT):
                        nc.tensor.matmul(
                            qp[:],
                            wq_sb[:, dt, et * P : (et + 1) * P],
                            xq[dt][:, qc * QC : (qc + 1) * QC],
                            start=(dt == 0),
                            stop=(dt == NDT - 1),
                            skip_group_check=True,
                        )
                    nc.vector.tensor_copy(qt_sb[et][:, qc * QC : (qc + 1) * QC], qp[:])

                # Q proj chunk 0 first: it gates the first scores and only
                # needs the xq stream, so it preempts K-proj leftovers.
                for et in range(2):
                    emit_qproj(0, et)

                # K proj in (et, qc) chunks, dt-inner, so kt_sb completes
                # incrementally (sweep 0 consumes et=0 chunks first).
                for et in range(2):
                    for qc in range(NQC):
                        kp = auxp.tile([P, QC], F32, name="kp", tag="aux")
                        for dt in range(NDT):
                            nc.tensor.matmul(
                                kp[:],
                                wk_sb[:, dt, et * P : (et + 1) * P],
                                xk[dt][:, qc * QC : (qc + 1) * QC],
                                start=(dt == 0),
                                stop=(dt == NDT - 1),
                                skip_group_check=True,
                            )
                        nc.vector.tensor_copy(
                            kt_sb[et][:, qc * QC : (qc + 1) * QC], kp[:]
                        )

                # V proj + Q proj chunks 1-3: demoted priority -> the
                # scheduler slots them into PE gaps under the exp stream.
                with tc.high_priority(offset=-DEMOTE):
                    for tt in range(NKT):
                        psv = vp.tile([P, EHC], F32, name="psv", tag="psv")
                        for dt in range(NDT):
                            nc.tensor.matmul(
                                psv[:],
                                xv[dt][:, tt * P : (tt + 1) * P],
                                wv_sb[:, dt, :],
                                start=(dt == 0),
                                stop=(dt == NDT - 1),
                                skip_group_check=True,
                            )
                        for h in range(NH):
                            nc.vector.tensor_copy(
                                v_sb[
                                    :,
                                    (tt * NH + h) * VW : (tt * NH + h) * VW + EH,
                                ],
                                psv[:, h * EH : (h + 1) * EH],
                            )
                    for qc in range(1, NQC):
                        for et in range(2):
                            emit_qproj(qc, et)

                heads_cat = [None] * (2 * NQC)

                def normalize(lh, pvt, hc):
                    # pvt rows 0:64 = head output, rows 64:128 = denominator
                    # (broadcast via the 64 ones columns), f32. Two aligned
                    # copies free the PSUM bank fast; reciprocal_approx_fast
                    # requires partition-0-aligned operands.
                    num = normp.tile([EH, QC], F32, name="num", tag="num")
                    nc.vector.tensor_copy(num[:], pvt[0:EH, :])
                    den = normp.tile([EH, QC], F32, name="den", tag="den")
                    nc.vector.tensor_copy(den[:], pvt[EH : 2 * EH, :])
                    rcp = normp.tile([EH, QC], F32, name="rcp", tag="rcp")
                    nc.vector.reciprocal_approx_fast(rcp[:], den[:])
                    nc.vector.tensor_mul(
                        hc[lh * EH : (lh + 1) * EH, :], num[:], rcp[:]
                    )

                def emit_wout(q4, pingpong=False):
                    # row-parallel partial: out^T[ot*128:+128, q chunk] from
                    # this core's 4 heads (contraction = 2 e-tiles of 128).
                    # pingpong alternates PSUM banks so the final chunk's
                    # mm->copy chain pipelines instead of serializing.
                    for ot in range(NDT):
                        if pingpong and ot % 2 == 1:
                            pso = pvp.tile([P, QC], F32, name="pso", tag="pv0")
                        else:
                            pso = auxp.tile([P, QC], F32, name="pso", tag="aux")
                        for ep2 in range(2):
                            nc.tensor.matmul(
                                pso[:],
                                wo3_sb[:, ep2, ot * P : (ot + 1) * P],
                                heads_cat[2 * q4 + ep2][:],
                                start=(ep2 == 0),
                                stop=(ep2 == 1),
                                skip_group_check=True,
                            )
                        ost = outstp.tile([P, QC], F32, name="ost", tag="ost")
                        nc.vector.tensor_copy(ost[:], pso[:])
                        nc.sync.dma_start(
                            outt[ot * P : (ot + 1) * P, q4 * QC : (q4 + 1) * QC],
                            ost[:],
                        )

                def voff(kt, h):
                    return (kt * NH + h) * VW

                for sweep in range(2 * NQC):
                    q4, ep = sweep // 2, sweep % 2
                    q0 = q4 * QC
                    hA, hB = 2 * ep, 2 * ep + 1
                    pv = [
                        pvp.tile([P, QC], F32, name=f"pv{lh}", tag=f"pv{lh}")
                        for lh in range(2)
                    ]
                    exring = [None] * NKT
                    for kt in range(NKT + 1):
                        if kt < NKT:
                            exq = expp.tile([P, 1024], BF, name="exq", tag="exq")
                            exring[kt] = exq
                            s_t = scorep.tile([P, 1024], F32, name="sq", tag="sq")
                            for lh in range(2):
                                po = lh * EH
                                nc.tensor.matmul(
                                    s_t[:, lh * QC : (lh + 1) * QC],
                                    kt_sb[ep][po : po + EH, kt * P : (kt + 1) * P],
                                    qt_sb[ep][po : po + EH, q0 : q0 + QC],
                                    start=True,
                                    stop=True,
                                )
                            nc.scalar.activation(
                                exq[:],
                                s_t[:],
                                mybir.ActivationFunctionType.Exp,
                                scale=float(1.0 / np.sqrt(EH)),
                            )
                        if kt >= 1:
                            pkt = kt - 1
                            for lh in range(2):
                                h = hA if lh == 0 else hB
                                nc.tensor.matmul(
                                    pv[lh][:],
                                    v_sb[:, voff(pkt, h) : voff(pkt, h) + P],
                                    exring[pkt][:, lh * QC : (lh + 1) * QC],
                                    start=(pkt == 0),
                                    stop=(pkt == NKT - 1),
                                    skip_group_check=True,
                                )
                    # normalize this sweep's two heads into a packed
                    # [128, QC] tile (head A rows 0:64, head B rows 64:128)
                    hc = hcp.tile([P, QC], BF, name="hc", tag="hc")
                    heads_cat[sweep] = hc
                    for lh in range(2):
                        normalize(lh, pv[lh], hc)
                    if ep == 1:
                        with tc.high_priority(offset=-DEMOTE):
                            emit_wout(q4, pingpong=(q4 == NQC - 1))

    nc.compile()
    return nc


def _sb_layout(w_dm_e, blocks):
    # [blocks*P, E] -> [P, blocks*E] partition-major so the device DMA into a
    # [P, blocks, E] SBUF tile is one contiguous transfer
    e = w_dm_e.shape[1]
    return np.ascontiguousarray(
        w_dm_e.reshape(blocks, P, e).transpose(1, 0, 2).reshape(P, blocks * e)
    )


def _prep_inputs(x_query, x_key, x_value, Wq, Wk, Wv, Wout):
    bf = ml_dtypes.bfloat16
    xt = {}
    for g in range(GROUPS):
        xt[g] = tuple(
            np.ascontiguousarray(np.asarray(x[g], dtype=np.float32).T).astype(bf)
            for x in (x_query, x_key, x_value)
        )
    in_maps = []
    for c in range(NCORES):
        g, r = c // 4, c % 4
        hs = slice(NH * r, NH * (r + 1))
        wq_c = _sb_layout(
            np.asarray(Wq[hs], dtype=np.float32).reshape(EHC, DM).T, NDT
        ).astype(bf)
        wk_c = _sb_layout(
            np.asarray(Wk[hs], dtype=np.float32).reshape(EHC, DM).T, NDT
        ).astype(bf)
        wv_c = _sb_layout(
            np.asarray(Wv[hs], dtype=np.float32).reshape(EHC, DM).T, NDT
        ).astype(bf)
        # Wout rows for my head dims: [EHC, DM], partition-major over 2 e-tiles
        wo3_c = _sb_layout(
            np.ascontiguousarray(
                np.asarray(Wout[:, EHC * r : EHC * (r + 1)], dtype=np.float32).T
            ),
            2,
        ).astype(bf)
        in_maps.append(
            {
                "xqt": xt[g][0],
                "xkt": xt[g][1],
                "xvt": xt[g][2],
                "wqt": wq_c,
                "wkt": wk_c,
                "wvt": wv_c,
                "wo3t": wo3_c,
            }
        )
    return in_maps


def kernel(x_query, x_key, x_value, Wq, Wk, Wv, Wout, _trace=False):
    global _cached_nc
    if _cached_nc is None:
        _cached_nc = build_nc()
    nc = _cached_nc

    in_maps = _prep_inputs(x_query, x_key, x_value, Wq, Wk, Wv, Wout)
    res = run_bass_kernel_spmd(nc, in_maps, list(range(NCORES)), trace=_trace)
    kernel.last_result = res

    out = np.empty((B, S, DM), dtype=np.float32)
    for g in range(GROUPS):
        acc = res.results[4 * g]["outt"].astype(np.float32).copy()
        for r in range(1, 4):
            acc += res.results[4 * g + r]["outt"]
        out[g] = acc.T
    return out
